# revision 41
# baseline (speedup 1.0000x reference)
"""EngineOrderFFT (Bluestein chirp-Z, fixed M=16384) Trainium2 kernel.

Strategy
--------
Pure data parallelism: batch dim B=64 split across 8 NeuronCores
(8 samples/core). Each sample's variable-length DFT (length n_b) is a
Bluestein transform with fixed FFT size M=16384 = 128*128; each
16384-point (i)FFT is a two-stage Cooley-Tukey factorization executed as
fp16 matmuls on the tensor engine:

  s1  (fwd stage 1)  data-as-lhsT, zero-padding -> K=64:   16mm/4096col
  s2  (fwd stage 2)  Karatsuba absorbed in PSUM accum:     24mm/6144col
  is1 (inv stage 1)  data-as-lhsT Karatsuba:               24mm/6144col
  is2 (inv stage 2)  data-as-lhsT Karatsuba, out [m1,m2]:  24mm/3072col

Between matmul stages sit three pointwise complex layers (fwd twiddle,
Fa*Fb, inv twiddle), computed as Karatsuba product planes on the DVE with
the P and Qn products fused into one instruction per half (the [re|im]
channel layout lines up with adjacent [Wr|Wni] table columns).  All
chirp/twiddle/Fb tables are stored un-replicated and broadcast across the
8 channel pages with 0-stride APs.

Scheduling: every PE phase, PSUM evacuation (ACT) and product layer (DVE)
is split into channel halves; the pipeline unit is a half-sample on a
quarter of PSUM, pair-interleaved, and each pair's is2 window (short,
product-gated) is filled with the next pair's s1 units, with the ACT
queue mirroring that order.  The a-planes (x*chirp) run on gpsimd with
6-deep input buffers so they stay well ahead.

|conv[k]| is computed on the host from the shipped re/im planes
(conv index k = m1 + 128*m2 with m1 the is2 output partition).
"""
import numpy as np

SF, RES, TS = 8192, 40, 1
B, L, C = 64, 8192, 8
M = 16384
NCORES = 8
SPC = B // NCORES  # samples per core

FBSCALE = 1.0 / 32.0
HSCALE = 1.0 / 16.0
KSCALE = 1.0 / 32.0  # HSCALE*KSCALE = (1/M) * (1/FBSCALE)

# ---------------------------------------------------------------------------
# constant tables (input-independent)
# ---------------------------------------------------------------------------


def _f16(x):
    return np.ascontiguousarray(x, dtype=np.float16)


def _build_const_tables():
    j = np.arange(128)
    D = np.exp(-2j * np.pi * np.outer(j, j) / 128.0)  # symmetric
    Dc = np.conj(D)
    Wt = np.exp(-2j * np.pi * np.outer(j, j) / M)  # fwd twiddle [n1,k2]
    W2 = np.conj(Wt)  # inv twiddle
    Dr, Di = D.real, D.imag
    Hr, Hi = (Dc * HSCALE).real, (Dc * HSCALE).imag
    Kr, Ki = (Dc * KSCALE).real[:, :64], (Dc * KSCALE).imag[:, :64]

    cols = []
    # chunk 1 (cols 0:1024): W twiddle (L1), F tables (s2)
    cols += [Wt.real, -Wt.imag, Wt.real + Wt.imag]  # _WR,_WNI,_WS [128,128]
    cols += [Dr + Di, Dr - Di, Di - Dr, -Di, Dr]  # F1,F2,F2n,F3,F4 [128,128]
    # chunk 2 (cols 1024:2560): H (is1), W2 (L3), K (is2)
    cols += [np.concatenate([Hr + Hi, Hi - Hr], 1)]  # H_P [128,256]
    cols += [np.concatenate([Hr - Hi, Hi + Hr], 1)]  # H_Q
    cols += [np.concatenate([-Hi, Hr], 1)]  # H_R
    cols += [W2.real, -W2.imag, W2.real + W2.imag]  # _W2R,_W2NI,_W2S
    cols += [np.concatenate([Kr + Ki, Ki - Kr], 1)]  # K_P [128,128]
    cols += [np.concatenate([Kr - Ki, Ki + Kr], 1)]  # K_Q
    cols += [np.concatenate([-Ki, Kr], 1)]  # K_R
    ca = _f16(np.concatenate(cols, axis=1))

    cb = _f16(
        np.concatenate([Dr[:64], Di[:64], -Di[:64], Dr[:64]], axis=1)
    )  # [64, 512] = Dtab1|Dtab2
    return ca, cb


# column offsets in ca
_WR, _WNI, _WS = 0, 128, 256
_F = [384, 512, 640, 768, 896]  # F1,F2,F2n,F3,F4
_HA, _HB, _HR = 1024, 1280, 1536
_W2R, _W2NI, _W2S = 1792, 1920, 2048
_KP, _KQ, _KR = 2176, 2304, 2432
CA1_COLS = 1024
CA_COLS = 2560

_CONST_CACHE = {}


def _consts():
    if "ca" not in _CONST_CACHE:
        ca, cb = _build_const_tables()
        assert ca.shape[1] == CA_COLS, ca.shape
        _CONST_CACHE["ca"] = ca
        _CONST_CACHE["cb"] = cb
    return _CONST_CACHE["ca"], _CONST_CACHE["cb"]


# ---------------------------------------------------------------------------
# device module
# ---------------------------------------------------------------------------

_MODULE_CACHE = {}


def _build_module():
    import concourse.bass as bass
    from concourse import mybir

    dt = mybir.dt
    NB = 2  # per-sample buffer depth

    nc = bass.Bass("TRN2", target_bir_lowering=False, debug=False)

    # packed input: [64, 0:1024] = x as [n2, c, n1], [64, 1024:1280] = chirp
    # (cos | -sin) as [n2, n1]
    xad = nc.dram_tensor("xad", [SPC, 64, 1280], dt.float16, kind="ExternalInput").ap()
    # Fb planes: [128, 0:512] = (Fbr,Fbr,-Fbi,-Fbi), [128, 512:640] = Fbr+Fbi
    fbd = nc.dram_tensor("fbd", [SPC, 128, 640], dt.float16, kind="ExternalInput").ap()
    cad = nc.dram_tensor("cad", [128, CA_COLS], dt.float16, kind="ExternalInput").ap()
    cbd = nc.dram_tensor("cbd", [64, 512], dt.float16, kind="ExternalInput").ap()
    # out: [SPC, 128(m1), 8ch * (re64|im64)(m2)] ; conv[m1+128*m2]
    outr = nc.dram_tensor("outr", [SPC, 128, 1024], dt.float16, kind="ExternalOutput").ap()

    ctx_list = []

    def sb(name, shape, dtype=None):
        t = nc.sbuf_tensor(name, shape, dtype or mybir.dt.float16)
        ap = t.__enter__()
        ctx_list.append(t)
        return ap

    def psum(name, shape):
        t = nc.psum_tensor(name, shape, mybir.dt.float32)
        ap = t.__enter__()
        ctx_list.append(t)
        return ap

    ca = sb("ca", [128, CA_COLS])
    cb = sb("cb", [64, 512])
    ND = 6  # input/a-plane buffer depth (Pool must run well ahead)
    xa_t = [sb(f"xa{i}", [64, 1280]) for i in range(ND)]
    fbR = [sb(f"fbR{i}", [128, 640]) for i in range(ND)]
    A_t = [sb(f"A{i}", [64, 2048]) for i in range(ND)]
    Yf = [sb(f"Yf{i}", [128, 2048]) for i in range(NB)]
    PQ1 = [sb(f"PQ1_{i}", [128, 2048]) for i in range(NB)]
    Rb = [sb(f"Rb{i}", [128, 1024]) for i in range(NB)]
    Ff = [sb(f"Ff{i}", [128, 2048]) for i in range(NB)]
    CRI = [sb(f"CRI{i}", [128, 2048]) for i in range(NB)]
    Sf = [sb(f"Sf{i}", [128, 2048]) for i in range(NB)]
    PQ3 = [sb(f"PQ3_{i}", [128, 2048]) for i in range(NB)]
    R3b = [sb(f"R3b{i}", [128, 1024]) for i in range(NB)]
    ob = [sb(f"ob{i}", [128, 1024]) for i in range(NB)]
    S1 = [sb(f"S1_{i}", [128, 1024]) for i in range(NB)]
    S3 = [sb(f"S3_{i}", [128, 1024]) for i in range(NB)]
    M1 = [sb(f"M1_{i}", [128, 1024]) for i in range(NB)]
    M2 = [sb(f"M2_{i}", [128, 1024]) for i in range(NB)]

    # two 4-bank psum regions; samples alternate regions by parity
    psR = [psum("psR0", [128, 2048]), psum("psR1", [128, 2048])]

    csem = nc.alloc_semaphore("csem")
    c2sem = nc.alloc_semaphore("c2sem")
    cbsem = nc.alloc_semaphore("cbsem")
    smp = [nc.alloc_semaphore(f"smp{i}") for i in range(SPC)]
    osem = [nc.alloc_semaphore(f"osem{i}") for i in range(SPC)]
    fsem = [nc.alloc_semaphore(f"fsem{i}") for i in range(SPC)]
    vsem = nc.alloc_semaphore("vsem")
    psem = nc.alloc_semaphore("psem")
    ssem = nc.alloc_semaphore("ssem")
    gsem = nc.alloc_semaphore("gsem")

    # ---- emission orders (pair-interleaved, half-split) and sem tables ----
    # Every evac / product layer / PE phase is split into channel halves
    # h=0 (ch 0-3) and h=1 (ch 4-7) so the evac->product->matmul chain per
    # half is ~1.7us instead of ~3us and PE stays fed.
    pairs = [(2 * p, 2 * p + 1) for p in range(SPC // 2)]

    pe_order = []   # (phase, s, h): half-sample units
    act_order = []  # (evac, s, h)
    dve_order = []  # (group, s), group in 0..2 (L1, CL, L3); halves inside
    gp_order = []   # (kind, s), kind 0=a-planes
    # Software pipeline: pair p's is2 window (short, product-gated units) is
    # filled with pair p+1's s1 units, and ACT mirrors that order so the ob
    # evacs (which free s1 regions) and the next Yf evacs (which feed the L1
    # chain) land just-in-time.
    for p, (sa, sb_) in enumerate(pairs):
        if p == 0:
            pe_order += [(0, sa, 0), (0, sa, 1), (0, sb_, 0), (0, sb_, 1)]
            act_order += [(0, sa, 0), (0, sa, 1), (0, sb_, 0), (0, sb_, 1)]
        for ph in (1, 2):
            pe_order += [(ph, sa, 0), (ph, sa, 1), (ph, sb_, 0), (ph, sb_, 1)]
            act_order += [(ph, sa, 0), (ph, sa, 1), (ph, sb_, 0), (ph, sb_, 1)]
        if p + 1 < len(pairs):
            na, nb_ = pairs[p + 1]
            pe_order += [
                (3, sa, 0), (3, sa, 1), (0, na, 0), (3, sb_, 0),
                (0, na, 1), (3, sb_, 1), (0, nb_, 0), (0, nb_, 1),
            ]
            act_order += [
                (3, sa, 0), (3, sa, 1), (0, na, 0), (3, sb_, 0),
                (0, na, 1), (3, sb_, 1), (0, nb_, 0), (0, nb_, 1),
            ]
        else:
            pe_order += [(3, sa, 0), (3, sa, 1), (3, sb_, 0), (3, sb_, 1)]
            act_order += [(3, sa, 0), (3, sa, 1), (3, sb_, 0), (3, sb_, 1)]
        for g in range(3):
            dve_order += [(g, sa), (g, sb_)]
        gp_order += [(0, sa), (0, sb_)]
    gp_order = [e for e in gp_order if e[1] >= 2]

    # PE increments psem once per half-phase, in emission order.
    PSEM = {}
    for i, key in enumerate(pe_order):
        PSEM[key] = i + 1
    SSEM = {}
    for i, key in enumerate(act_order):
        SSEM[key] = i + 1
    GSEM = {}
    g = 0
    for kind, s in gp_order:
        g += 2
        GSEM[(kind, s)] = g
    # DVE op positions per (group, half): PQ(+1), S(+2), R(+3)
    VSEM = {}
    VOP = {}
    v = 8  # 8 startup a-plane half-ops on DVE (samples 0,1)
    for grp, s in dve_order:
        for h in range(2):
            for k in range(1, 4):
                VOP[(grp, s, h, k)] = v + 3 * h + k
        v += 6
        VSEM[(grp, s)] = v

    AluOp = mybir.AluOpType

    def bcast8(tab):
        """[P,128] table -> [P, 8, 128] zero-stride channel broadcast."""
        p = tab.shape[0]
        return tab.rearrange("p (o u) -> p o u", o=1).broadcast_to((p, 8, 128))

    with nc.Block() as block:

        @block.sync
        def _(sync):
            def emit_in(s):
                b = s % NB
                b4 = s % ND
                if s >= ND:
                    if s - ND < 2:
                        sync.wait_ge(vsem, 4 * (s - ND + 1))
                    else:
                        sync.wait_ge(gsem, GSEM[(0, s - ND)])
                if s >= ND:
                    sync.wait_ge(vsem, VSEM[(1, s - ND)])
                sync.dma_start(xa_t[b4][:], xad[s]).then_inc(smp[s], 16)
                sync.dma_start(fbR[s % ND][:], fbd[s]).then_inc(fsem[s], 16)

            def emit_out(s):
                b = s % NB
                for jj in range(2):
                    sync.wait_ge(ssem, SSEM[(3, s, jj)])
                    sync.dma_start(
                        outr[s][:, 512 * jj : 512 * jj + 512],
                        ob[b][:, 512 * jj : 512 * jj + 512],
                    ).then_inc(osem[s], 16)

            # startup: xa0 first so the a-planes (and s1) start ASAP;
            # const tables go down the scalar engine's DMA queue in parallel
            sync.dma_start(xa_t[0][:], xad[0]).then_inc(smp[0], 16)
            sync.dma_start(fbR[0][:], fbd[0]).then_inc(fsem[0], 16)
            emit_in(1)
            for s_ in range(2, ND):
                emit_in(s_)
            for s in range(SPC):
                if s + ND < SPC:
                    emit_in(s + ND)
                emit_out(s)

        @block.gpsimd
        def _(gp):
            for kind, s in gp_order:
                b4 = s % ND
                gp.wait_ge(smp[s], 16)
                if s >= ND:
                    gp.wait_ge(psem, PSEM[(0, s - ND, 1)])  # A_t[b4] free
                xv = xa_t[b4][:, 0:1024].rearrange("p (c n) -> p c n", c=C)
                nc.gpsimd.tensor_tensor(
                    A_t[b4][:, 0:1024].rearrange("p (c n) -> p c n", c=C),
                    xv,
                    bcast8(xa_t[b4][:, 1024:1152]),
                    AluOp.mult,
                ).then_inc(gsem, 1)
                nc.gpsimd.tensor_tensor(
                    A_t[b4][:, 1024:2048].rearrange("p (c n) -> p c n", c=C),
                    xv,
                    bcast8(xa_t[b4][:, 1152:1280]),
                    AluOp.mult,
                ).then_inc(gsem, 1)

        @block.vector
        def _(vector):
            def chpages(ap):
                v_ = ap.rearrange("p (c u) -> p c u", c=C)
                return v_[:, :, 0:128], v_[:, :, 128:256]

            def prpages(ap):
                # s2 output layout: 4 q-blocks of (re 2ch*128 | im 2ch*128)
                v_ = ap.rearrange("p (q r u) -> p q r u", q=4, r=2)
                return v_[:, :, 0, :], v_[:, :, 1, :]  # [128, 4, 256] each

            def flat8(ap):
                return ap.rearrange("p (c u) -> p c u", c=C)

            def flat4(ap):
                return ap.rearrange("p (q u) -> p q u", q=4)

            def bcast4x2(tab):
                # [128,128] -> [128, 4, 2, 128] for the q-block × 2ch layout
                return tab.rearrange("p (o q u) -> p o q u", o=1, q=1).broadcast_to(
                    (128, 4, 2, 128)
                )

            def bch4(tab):
                return tab.rearrange("p (o u) -> p o u", o=1).broadcast_to(
                    (64, 4, 128)
                )

            for s0 in (0, 1):
                vector.wait_ge(smp[s0], 16)
                xv = xa_t[s0][:, 0:1024].rearrange("p (c n) -> p c n", c=C)
                for hh in range(2):
                    c4 = slice(4 * hh, 4 * hh + 4)
                    nc.vector.tensor_tensor(
                        A_t[s0][:, 0:1024].rearrange("p (c n) -> p c n", c=C)[
                            :, c4, :
                        ],
                        xv[:, c4, :],
                        bch4(xa_t[s0][:, 1024:1152]),
                        AluOp.mult,
                    ).then_inc(vsem, 1)
                    nc.vector.tensor_tensor(
                        A_t[s0][:, 1024:2048].rearrange("p (c n) -> p c n", c=C)[
                            :, c4, :
                        ],
                        xv[:, c4, :],
                        bch4(xa_t[s0][:, 1152:1280]),
                        AluOp.mult,
                    ).then_inc(vsem, 1)
            def bc_h(tab, n, w):
                # [128,w] table -> [128, n, w] zero-stride broadcast
                return tab.rearrange("p (o u) -> p o u", o=1).broadcast_to(
                    (128, n, w)
                )

            first_dve = [True]
            for grp, s in dve_order:
                if first_dve[0]:
                    vector.wait_ge(csem, 16)
                    first_dve[0] = False
                    first_l3 = [True]
                b = s % NB
                if grp == 0:
                    # L1 (fwd twiddle, Karatsuba planes) from Yf, by halves
                    if s >= NB:
                        vector.wait_ge(psem, PSEM[(1, s - NB, 1)])  # bufs free
                    yv = Yf[b][:].rearrange("p (c u) -> p c u", c=C)
                    pv = PQ1[b][:].rearrange("p (c u) -> p c u", c=C)
                    for h in range(2):
                        vector.wait_ge(ssem, SSEM[(0, s, h)])
                        c4 = slice(4 * h, 4 * h + 4)
                        nc.vector.tensor_tensor(
                            pv[:, c4, :], yv[:, c4, :],
                            bc_h(ca[:, 0:256], 4, 256), AluOp.mult,
                        ).then_inc(vsem, 1)  # [P|Qn] per channel
                        nc.vector.tensor_tensor(
                            flat8(S1[b][:])[:, c4, :],
                            yv[:, c4, 0:128], yv[:, c4, 128:256], AluOp.add,
                        ).then_inc(vsem, 1)
                        vector.wait_ge(vsem, VOP[(grp, s, h, 2)])  # S1h drained
                        nc.vector.tensor_tensor(
                            flat8(Rb[b][:])[:, c4, :],
                            flat8(S1[b][:])[:, c4, :],
                            bc_h(ca[:, _WS : _WS + 128], 4, 128), AluOp.mult,
                        ).then_inc(vsem, 1)
                elif grp == 1:
                    # C-layer (Fa o Fb, Karatsuba planes) from Ff, by halves
                    vector.wait_ge(fsem[s], 16)
                    if s >= NB:
                        vector.wait_ge(psem, PSEM[(2, s - NB, 1)])  # bufs free
                    fv = Ff[b][:].rearrange("p (q u) -> p q u", q=4)
                    cv = CRI[b][:].rearrange("p (q u) -> p q u", q=4)
                    for h in range(2):
                        vector.wait_ge(ssem, SSEM[(1, s, h)])
                        q2 = slice(2 * h, 2 * h + 2)
                        nc.vector.tensor_tensor(
                            cv[:, q2, :], fv[:, q2, :],
                            bc_h(fbR[s % ND][:, 0:512], 2, 512), AluOp.mult,
                        ).then_inc(vsem, 1)  # [CR 2ch | CI 2ch] per q
                        nc.vector.tensor_tensor(
                            flat4(M1[b][:])[:, q2, :],
                            fv[:, q2, 0:256], fv[:, q2, 256:512], AluOp.add,
                        ).then_inc(vsem, 1)
                        vector.wait_ge(vsem, VOP[(grp, s, h, 2)])  # M1h drained
                        nc.vector.tensor_tensor(
                            flat4(M2[b][:])[:, q2, :].rearrange(
                                "p q (c u) -> p q c u", c=2
                            ),
                            flat4(M1[b][:])[:, q2, :].rearrange(
                                "p q (c u) -> p q c u", c=2
                            ),
                            fbR[s % ND][:, 512:640].rearrange(
                                "p (o q u) -> p o q u", o=1, q=1
                            ).broadcast_to((128, 2, 2, 128)),
                            AluOp.mult,
                        ).then_inc(vsem, 1)
                else:
                    # L3 (inv twiddle, Karatsuba planes) from Sf, by halves
                    if first_l3[0]:
                        vector.wait_ge(c2sem, 16)
                        first_l3[0] = False
                    if s >= NB:
                        vector.wait_ge(psem, PSEM[(3, s - NB, 1)])  # bufs free
                    sv = Sf[b][:].rearrange("p (c u) -> p c u", c=C)
                    p3v = PQ3[b][:].rearrange("p (c u) -> p c u", c=C)
                    for h in range(2):
                        vector.wait_ge(ssem, SSEM[(2, s, h)])
                        c4 = slice(4 * h, 4 * h + 4)
                        nc.vector.tensor_tensor(
                            p3v[:, c4, :], sv[:, c4, :],
                            bc_h(ca[:, _W2R : _W2R + 256], 4, 256), AluOp.mult,
                        ).then_inc(vsem, 1)
                        nc.vector.tensor_tensor(
                            flat8(S3[b][:])[:, c4, :],
                            sv[:, c4, 0:128], sv[:, c4, 128:256], AluOp.add,
                        ).then_inc(vsem, 1)
                        vector.wait_ge(vsem, VOP[(grp, s, h, 2)])  # S3h drained
                        nc.vector.tensor_tensor(
                            flat8(R3b[b][:])[:, c4, :],
                            flat8(S3[b][:])[:, c4, :],
                            bc_h(ca[:, _W2S : _W2S + 128], 4, 128), AluOp.mult,
                        ).then_inc(vsem, 1)

        @block.tensor
        def _(tensor):
            mm = nc.tensor.matmul
            first_pe = [True]
            first_is1 = [True]

            def phase_s1(s, h):
                rg = psR[s % 2][:, 1024 * h : 1024 * h + 1024]
                if first_pe[0]:
                    tensor.wait_ge(cbsem, 16)  # cb loaded
                    first_pe[0] = False
                if s < 2:
                    tensor.wait_ge(vsem, 4 * s + 2 * (h + 1))  # startup planes
                elif h == 0:
                    tensor.wait_ge(gsem, GSEM[(0, s)])
                if s >= NB:
                    # region free once ob evac half h of s-NB done
                    tensor.wait_ge(ssem, SSEM[(3, s - NB, h)])
                b4 = s % ND
                for c in range(4 * h, 4 * h + 4):
                    o = rg[:, 256 * (c - 4 * h) : 256 * (c - 4 * h) + 256]
                    mm(
                        o,
                        A_t[b4][:, 128 * c : 128 * c + 128],
                        cb[:, 0:256],
                        start=True,
                        stop=False,
                    )
                    i = mm(
                        o,
                        A_t[b4][:, 1024 + 128 * c : 1024 + 128 * c + 128],
                        cb[:, 256:512],
                        start=False,
                        stop=True,
                    )
                    if c % 4 == 3:
                        i.then_inc(psem, 1)

            def phase_s2(s, h):
                b = s % NB
                rg = psR[s % 2][:, 1024 * h : 1024 * h + 1024]
                if h == 0:
                    tensor.wait_ge(csem, 16)  # ca chunk1 loaded
                def pq1(q, r):
                    # plane r (0=P, 1=Qn) of channels 2q, 2q+1 from PQ1
                    return PQ1[b][:, 512 * q : 512 * q + 512].rearrange(
                        "p (c r u) -> p c r u", c=2, r=2
                    )[:, :, r, :]

                srcs = [
                    (0, _F[0], 0, True, False, 1),
                    (1, _F[0], 256, False, False, None),
                    (1, _F[1], 0, False, False, None),
                    (0, _F[2], 256, False, False, None),
                    (2, _F[3], 0, False, False, 3),
                    (2, _F[4], 256, False, True, None),
                ]
                for wi, (pr, fofs, oofs, st, sp, wk) in enumerate(srcs):
                    if wk is not None:
                        tensor.wait_ge(vsem, VOP[(0, s, h, wk)])
                    for ql in range(2):
                        q = 2 * h + ql
                        rhs = (
                            Rb[b][:, 256 * q : 256 * q + 256]
                            if pr == 2
                            else pq1(q, pr)
                        )
                        i = mm(
                            rg[:, 512 * ql + oofs : 512 * ql + oofs + 256],
                            ca[:, fofs : fofs + 128],
                            rhs,
                            start=st,
                            stop=sp,
                        )
                        if wi == 5 and ql == 1:
                            i.then_inc(psem, 1)

            def phase_is1(s, h):
                b = s % NB
                rg = psR[s % 2][:, 1024 * h : 1024 * h + 1024]
                if first_is1[0]:
                    tensor.wait_ge(c2sem, 16)  # H tables in the 2nd const DMA
                    first_is1[0] = False
                cs = range(4 * h, 4 * h + 4)
                tensor.wait_ge(vsem, VOP[(1, s, h, 1)])  # [CR|CI]h ready
                for c in cs:
                    # even channel opens its bank; odd writes the other half
                    mm(
                        rg[:, 256 * (c % 4) : 256 * (c % 4) + 256],
                        CRI[b][
                            :,
                            512 * (c // 2)
                            + 128 * (c % 2) : 512 * (c // 2)
                            + 128 * (c % 2)
                            + 128,
                        ],
                        ca[:, _HA : _HA + 256],
                        start=(c % 2 == 0),
                        stop=False,
                    )
                for c in cs:
                    mm(
                        rg[:, 256 * (c % 4) : 256 * (c % 4) + 256],
                        CRI[b][
                            :,
                            512 * (c // 2)
                            + 256
                            + 128 * (c % 2) : 512 * (c // 2)
                            + 256
                            + 128 * (c % 2)
                            + 128,
                        ],
                        ca[:, _HB : _HB + 256],
                        start=False,
                        stop=False,
                    )
                tensor.wait_ge(vsem, VOP[(1, s, h, 3)])  # M2h ready
                for c in cs:
                    i = mm(
                        rg[:, 256 * (c % 4) : 256 * (c % 4) + 256],
                        M2[b][:, 128 * c : 128 * c + 128],
                        ca[:, _HR : _HR + 256],
                        start=False,
                        stop=(c % 2 == 1),
                    )
                    if c % 4 == 3:
                        i.then_inc(psem, 1)

            def phase_is2(s, h):
                b = s % NB
                rg = psR[s % 2][:, 1024 * h : 1024 * h + 1024]
                cs = range(4 * h, 4 * h + 4)
                tensor.wait_ge(vsem, VOP[(2, s, h, 1)])  # [P3|Q3n]h ready
                for c in cs:
                    mm(
                        rg[:, 128 * (c % 4) : 128 * (c % 4) + 128],
                        PQ3[b][:, 256 * c : 256 * c + 128],
                        ca[:, _KP : _KP + 128],
                        start=(c % 4 == 0),
                        stop=False,
                    )
                for c in cs:
                    mm(
                        rg[:, 128 * (c % 4) : 128 * (c % 4) + 128],
                        PQ3[b][:, 256 * c + 128 : 256 * c + 256],
                        ca[:, _KQ : _KQ + 128],
                        start=False,
                        stop=False,
                    )
                tensor.wait_ge(vsem, VOP[(2, s, h, 3)])  # R3bh ready
                for c in cs:
                    i = mm(
                        rg[:, 128 * (c % 4) : 128 * (c % 4) + 128],
                        R3b[b][:, 128 * c : 128 * c + 128],
                        ca[:, _KR : _KR + 128],
                        start=False,
                        stop=(c % 4 == 3),
                    )
                    if c % 4 == 3:
                        i.then_inc(psem, 1)

            phase_fns = [phase_s1, phase_s2, phase_is1, phase_is2]
            for ph, s, h in pe_order:
                phase_fns[ph](s, h)

        @block.scalar
        def _(scalar):
            nc.scalar.dma_start(cb[:], cbd[:]).then_inc(cbsem, 16)
            nc.scalar.dma_start(ca[:, 0:CA1_COLS], cad[:, 0:CA1_COLS]).then_inc(
                csem, 16
            )
            nc.scalar.dma_start(ca[:, CA1_COLS:], cad[:, CA1_COLS:]).then_inc(
                c2sem, 16
            )
            for ph, s, h in act_order:
                b = s % NB
                ps = psR[s % 2]
                scalar.wait_ge(psem, PSEM[(ph, s, h)])
                if ph == 3:
                    if s >= NB and h == 0:
                        scalar.wait_ge(osem[s - NB], 32)
                    nc.scalar.copy(
                        ob[b][:, 512 * h : 512 * h + 512],
                        ps[:, 1024 * h : 1024 * h + 512],
                    ).then_inc(ssem, 1)
                else:
                    dst = [Yf, Ff, Sf][ph][b]
                    o = slice(1024 * h, 1024 * h + 1024)
                    nc.scalar.copy(dst[:, o], ps[:, o]).then_inc(ssem, 1)

    for t in reversed(ctx_list):
        t.__exit__(None, None, None)

    return nc


def _get_module():
    if "nc" not in _MODULE_CACHE:
        _MODULE_CACHE["nc"] = _build_module()
    return _MODULE_CACHE["nc"]


# ---------------------------------------------------------------------------
# host side
# ---------------------------------------------------------------------------


def _host_tables(rpm):
    """Per-sample chirp tables + Fb planes (un-replicated)."""
    pad = np.floor((RES * 60.0 / rpm.astype(np.float64) - TS) * SF).astype(np.int64)
    n_arr = L + pad
    t = np.arange(L, dtype=np.int64)
    m = np.arange(M, dtype=np.int64)
    mm = np.minimum(m, M - m)

    ach = np.empty((B, 256), np.float16)   # per n2-row: [cos 128 | -sin 128]
    fbp = np.empty((B, 128, 640), np.float16)
    for b in range(B):
        n = int(n_arr[b])
        two_n = 2 * n
        ph = np.pi * ((t * t) % two_n) / n
        cosv = np.cos(ph).astype(np.float16).reshape(64, 128)
        nsin = (-np.sin(ph)).astype(np.float16).reshape(64, 128)
        ach[b] = 0  # unused filler; real packing happens in kernel()
        _ACH_COS[b] = cosv
        _ACH_NSIN[b] = nsin
        phb = np.pi * ((mm * mm) % two_n) / n
        Fb = np.fft.fft(np.exp(1j * phb)).reshape(128, 128) * FBSCALE
        fr = Fb.real.astype(np.float16)
        fni = (-Fb.imag).astype(np.float16)
        fs = (Fb.real + Fb.imag).astype(np.float16)
        fbp[b] = np.concatenate([fr, fr, fni, fni, fs], axis=1)
    return fbp


_ACH_COS = np.empty((B, 64, 128), np.float16)
_ACH_NSIN = np.empty((B, 64, 128), np.float16)


LAST_EXEC_WALL_NS = [None]


def kernel(inputs, rpm):
    inputs = np.ascontiguousarray(inputs, dtype=np.float32)  # [B, L, C]
    rpm = np.ascontiguousarray(rpm, dtype=np.float32)

    ca, cb = _consts()
    fbp = _host_tables(rpm)
    # pack per-sample [64, 1280]: x as [n2, c, n1] cols 0:1024, chirp cols
    # 1024:1280 = [cos | -sin]
    xa = np.empty((B, 64, 1280), np.float16)
    xv = inputs.reshape(B, 64, 128, C).transpose(0, 1, 3, 2)  # [B, n2, c, n1]
    xa[:, :, 0:1024] = xv.reshape(B, 64, 1024).astype(np.float16)
    xa[:, :, 1024:1152] = _ACH_COS
    xa[:, :, 1152:1280] = _ACH_NSIN

    nc = _get_module()
    in_maps = []
    for g in range(NCORES):
        s0 = g * SPC
        in_maps.append(
            {
                "xad": xa[s0 : s0 + SPC],
                "fbd": fbp[s0 : s0 + SPC],
                "cad": ca,
                "cbd": cb,
            }
        )

    import time

    from concourse.bass_utils import run_bass_kernel_spmd

    t0 = time.perf_counter_ns()
    res = run_bass_kernel_spmd(nc, in_maps, list(range(NCORES)))
    LAST_EXEC_WALL_NS[0] = time.perf_counter_ns() - t0

    out = np.empty((B, L, C), np.float32)
    for g in range(NCORES):
        planes = np.asarray(res.results[g]["outr"], np.float32)  # [SPC, 128, 1024]
        arr = planes.reshape(SPC, 128, C, 2, 64)  # [s, m1, c, re|im, m2]
        mag = np.hypot(arr[:, :, :, 0, :], arr[:, :, :, 1, :])  # [s, m1, c, m2]
        # conv index k = m1 + 128*m2  ->  out[s, k, c]
        out[g * SPC : (g + 1) * SPC] = (
            mag.transpose(0, 3, 1, 2).reshape(SPC, L, C)
        )
    return out


# revision 53
# speedup vs baseline: 1.0056x; 1.0056x over previous
"""EngineOrderFFT (Bluestein chirp-Z, fixed M=16384) Trainium2 kernel.

Strategy
--------
Pure data parallelism: batch dim B=64 split across 8 NeuronCores
(8 samples/core). Each sample's variable-length DFT (length n_b) is a
Bluestein transform with fixed FFT size M=16384 = 128*128; each
16384-point (i)FFT is a two-stage Cooley-Tukey factorization executed as
fp16 matmuls on the tensor engine:

  s1  (fwd stage 1)  data-as-lhsT, zero-padding -> K=64:   16mm/4096col
  s2  (fwd stage 2)  Karatsuba absorbed in PSUM accum:     24mm/6144col
  is1 (inv stage 1)  data-as-lhsT Karatsuba:               24mm/6144col
  is2 (inv stage 2)  data-as-lhsT Karatsuba, out [m1,m2]:  24mm/3072col

Between matmul stages sit three pointwise complex layers (fwd twiddle,
Fa*Fb, inv twiddle), computed as Karatsuba product planes on the DVE with
the P and Qn products fused into one instruction per half (the [re|im]
channel layout lines up with adjacent [Wr|Wni] table columns).  All
chirp/twiddle/Fb tables are stored un-replicated and broadcast across the
8 channel pages with 0-stride APs.

Scheduling: every PE phase, PSUM evacuation (ACT) and product layer (DVE)
is split into channel halves; the pipeline unit is a half-sample on a
quarter of PSUM, pair-interleaved, and each pair's is2 window (short,
product-gated) is filled with the next pair's s1 units, with the ACT
queue mirroring that order.  The a-planes (x*chirp) run on gpsimd with
6-deep input buffers so they stay well ahead.

|conv[k]| is computed on the host from the shipped re/im planes
(conv index k = m1 + 128*m2 with m1 the is2 output partition).
"""
import numpy as np

SF, RES, TS = 8192, 40, 1
B, L, C = 64, 8192, 8
M = 16384
NCORES = 8
SPC = B // NCORES  # samples per core

FBSCALE = 1.0 / 32.0
HSCALE = 1.0 / 16.0
KSCALE = 1.0 / 32.0  # HSCALE*KSCALE = (1/M) * (1/FBSCALE)

# ---------------------------------------------------------------------------
# constant tables (input-independent)
# ---------------------------------------------------------------------------


def _f16(x):
    return np.ascontiguousarray(x, dtype=np.float16)


def _build_const_tables():
    j = np.arange(128)
    D = np.exp(-2j * np.pi * np.outer(j, j) / 128.0)  # symmetric
    Dc = np.conj(D)
    Wt = np.exp(-2j * np.pi * np.outer(j, j) / M)  # fwd twiddle [n1,k2]
    W2 = np.conj(Wt)  # inv twiddle
    Dr, Di = D.real, D.imag
    Hr, Hi = (Dc * HSCALE).real, (Dc * HSCALE).imag
    Kr, Ki = (Dc * KSCALE).real[:, :64], (Dc * KSCALE).imag[:, :64]

    cols = []
    # chunk 1 (cols 0:1024): W twiddle (L1), F tables (s2)
    cols += [Wt.real, -Wt.imag, Wt.real + Wt.imag]  # _WR,_WNI,_WS [128,128]
    cols += [Dr + Di, Dr - Di, Di - Dr, -Di, Dr]  # F1,F2,F2n,F3,F4 [128,128]
    # chunk 2 (cols 1024:2560): H (is1), W2 (L3), K (is2)
    cols += [np.concatenate([Hr + Hi, Hi - Hr], 1)]  # H_P [128,256]
    cols += [np.concatenate([Hr - Hi, Hi + Hr], 1)]  # H_Q
    cols += [np.concatenate([-Hi, Hr], 1)]  # H_R
    cols += [W2.real, -W2.imag, W2.real + W2.imag]  # _W2R,_W2NI,_W2S
    cols += [np.concatenate([Kr + Ki, Ki - Kr], 1)]  # K_P [128,128]
    cols += [np.concatenate([Kr - Ki, Ki + Kr], 1)]  # K_Q
    cols += [np.concatenate([-Ki, Kr], 1)]  # K_R
    ca = _f16(np.concatenate(cols, axis=1))

    cb = _f16(
        np.concatenate([Dr[:64], Di[:64], -Di[:64], Dr[:64]], axis=1)
    )  # [64, 512] = Dtab1|Dtab2
    return ca, cb


# column offsets in ca
_WR, _WNI, _WS = 0, 128, 256
_F = [384, 512, 640, 768, 896]  # F1,F2,F2n,F3,F4
_HA, _HB, _HR = 1024, 1280, 1536
_W2R, _W2NI, _W2S = 1792, 1920, 2048
_KP, _KQ, _KR = 2176, 2304, 2432
CA1_COLS = 1024
CA_COLS = 2560

_CONST_CACHE = {}


def _consts():
    if "ca" not in _CONST_CACHE:
        ca, cb = _build_const_tables()
        assert ca.shape[1] == CA_COLS, ca.shape
        _CONST_CACHE["ca"] = ca
        _CONST_CACHE["cb"] = cb
    return _CONST_CACHE["ca"], _CONST_CACHE["cb"]


# ---------------------------------------------------------------------------
# device module
# ---------------------------------------------------------------------------

_MODULE_CACHE = {}


def _build_module():
    import concourse.bass as bass
    from concourse import mybir

    dt = mybir.dt
    NB = 2  # per-sample buffer depth

    nc = bass.Bass("TRN2", target_bir_lowering=False, debug=False)

    # packed input: [64, 0:1024] = x as [n2, c, n1], [64, 1024:1280] = chirp
    # (cos | -sin) as [n2, n1]
    xad = nc.dram_tensor("xad", [SPC, 64, 1280], dt.float16, kind="ExternalInput").ap()
    # Fb planes: [128, 0:512] = (Fbr,Fbr,-Fbi,-Fbi), [128, 512:640] = Fbr+Fbi
    fbd = nc.dram_tensor("fbd", [SPC, 128, 640], dt.float16, kind="ExternalInput").ap()
    cad = nc.dram_tensor("cad", [128, CA_COLS], dt.float16, kind="ExternalInput").ap()
    cbd = nc.dram_tensor("cbd", [64, 512], dt.float16, kind="ExternalInput").ap()
    # out: [SPC, 128(m1), 8ch * (re64|im64)(m2)] ; conv[m1+128*m2]
    outr = nc.dram_tensor("outr", [SPC, 128, 1024], dt.float16, kind="ExternalOutput").ap()

    ctx_list = []

    def sb(name, shape, dtype=None):
        t = nc.sbuf_tensor(name, shape, dtype or mybir.dt.float16)
        ap = t.__enter__()
        ctx_list.append(t)
        return ap

    def psum(name, shape):
        t = nc.psum_tensor(name, shape, mybir.dt.float32)
        ap = t.__enter__()
        ctx_list.append(t)
        return ap

    ca = sb("ca", [128, CA_COLS])
    cb = sb("cb", [64, 512])
    ND = 6  # input/a-plane buffer depth (Pool must run well ahead)
    xa_t = [sb(f"xa{i}", [64, 1280]) for i in range(ND)]
    fbR = [sb(f"fbR{i}", [128, 640]) for i in range(ND)]
    A_t = [sb(f"A{i}", [64, 2048]) for i in range(ND)]
    Yf = [sb(f"Yf{i}", [128, 2048]) for i in range(NB)]
    PQ1 = [sb(f"PQ1_{i}", [128, 2048]) for i in range(NB)]
    Rb = [sb(f"Rb{i}", [128, 1024]) for i in range(NB)]
    Ff = [sb(f"Ff{i}", [128, 2048]) for i in range(NB)]
    CRI = [sb(f"CRI{i}", [128, 2048]) for i in range(NB)]
    Sf = [sb(f"Sf{i}", [128, 2048]) for i in range(NB)]
    PQ3 = [sb(f"PQ3_{i}", [128, 2048]) for i in range(NB)]
    R3b = [sb(f"R3b{i}", [128, 1024]) for i in range(NB)]
    ob = [sb(f"ob{i}", [128, 1024]) for i in range(NB)]
    S1 = [sb(f"S1_{i}", [128, 1024]) for i in range(NB)]
    S3 = [sb(f"S3_{i}", [128, 1024]) for i in range(NB)]
    M1 = [sb(f"M1_{i}", [128, 1024]) for i in range(NB)]
    M2 = [sb(f"M2_{i}", [128, 1024]) for i in range(NB)]

    # two 4-bank psum regions; samples alternate regions by parity
    psR = [psum("psR0", [128, 2048]), psum("psR1", [128, 2048])]

    csem = nc.alloc_semaphore("csem")
    c2sem = nc.alloc_semaphore("c2sem")
    cbsem = nc.alloc_semaphore("cbsem")
    smp = [nc.alloc_semaphore(f"smp{i}") for i in range(SPC)]
    osem = [nc.alloc_semaphore(f"osem{i}") for i in range(SPC)]
    fsem = [nc.alloc_semaphore(f"fsem{i}") for i in range(SPC)]
    vsem = nc.alloc_semaphore("vsem")
    psem = nc.alloc_semaphore("psem")
    ssem = nc.alloc_semaphore("ssem")
    gsem = nc.alloc_semaphore("gsem")

    # ---- emission orders (pair-interleaved, half-split) and sem tables ----
    # Every evac / product layer / PE phase is split into channel halves
    # h=0 (ch 0-3) and h=1 (ch 4-7) so the evac->product->matmul chain per
    # half is ~1.7us instead of ~3us and PE stays fed.
    pairs = [(2 * p, 2 * p + 1) for p in range(SPC // 2)]

    pe_order = []   # (phase, s, h): half-sample units
    act_order = []  # (evac, s, h)
    dve_order = []  # (group, s), group in 0..2 (L1, CL, L3); halves inside
    gp_order = []   # (kind, s), kind 0=a-planes
    # Software pipeline: pair p's is2 window (short, product-gated units) is
    # filled with pair p+1's s1 units, and ACT mirrors that order so the ob
    # evacs (which free s1 regions) and the next Yf evacs (which feed the L1
    # chain) land just-in-time.
    for p, (sa, sb_) in enumerate(pairs):
        if p == 0:
            pe_order += [(0, sa, 0), (0, sa, 1), (0, sb_, 0), (0, sb_, 1)]
            act_order += [(0, sa, 0), (0, sa, 1), (0, sb_, 0), (0, sb_, 1)]
        for ph in (1, 2):
            pe_order += [(ph, sa, 0), (ph, sa, 1), (ph, sb_, 0), (ph, sb_, 1)]
            act_order += [(ph, sa, 0), (ph, sa, 1), (ph, sb_, 0), (ph, sb_, 1)]
        if p + 1 < len(pairs):
            na, nb_ = pairs[p + 1]
            pe_order += [
                (3, sa, 0), (3, sa, 1), (0, na, 0), (3, sb_, 0),
                (0, na, 1), (3, sb_, 1), (0, nb_, 0), (0, nb_, 1),
            ]
            act_order += [
                (3, sa, 0), (3, sa, 1), (0, na, 0), (3, sb_, 0),
                (0, na, 1), (3, sb_, 1), (0, nb_, 0), (0, nb_, 1),
            ]
        else:
            pe_order += [(3, sa, 0), (3, sa, 1), (3, sb_, 0), (3, sb_, 1)]
            act_order += [(3, sa, 0), (3, sa, 1), (3, sb_, 0), (3, sb_, 1)]
        for g in range(3):
            dve_order += [(g, sa), (g, sb_)]
        gp_order += [(0, sa), (0, sb_)]
    gp_order = [e for e in gp_order if e[1] >= 2]

    # PE increments psem once per half-phase, in emission order.
    PSEM = {}
    for i, key in enumerate(pe_order):
        PSEM[key] = i + 1
    SSEM = {}
    for i, key in enumerate(act_order):
        SSEM[key] = i + 1
    GSEM = {}
    g = 0
    for kind, s in gp_order:
        g += 2
        GSEM[(kind, s)] = g
    # DVE op positions per (group, half): PQ(+1), S(+2), R(+3)
    VSEM = {}
    VOP = {}
    v = 8  # 8 startup a-plane half-ops on DVE (samples 0,1)
    for grp, s in dve_order:
        for h in range(2):
            for k in range(1, 4):
                VOP[(grp, s, h, k)] = v + 3 * h + k
        v += 6
        VSEM[(grp, s)] = v

    AluOp = mybir.AluOpType

    def bcast8(tab):
        """[P,128] table -> [P, 8, 128] zero-stride channel broadcast."""
        p = tab.shape[0]
        return tab.rearrange("p (o u) -> p o u", o=1).broadcast_to((p, 8, 128))

    with nc.Block() as block:

        @block.sync
        def _(sync):
            def emit_in(s):
                b = s % NB
                b4 = s % ND
                if s >= ND:
                    if s - ND < 2:
                        sync.wait_ge(vsem, 4 * (s - ND + 1))
                    else:
                        sync.wait_ge(gsem, GSEM[(0, s - ND)])
                if s >= ND:
                    sync.wait_ge(vsem, VSEM[(1, s - ND)])
                sync.dma_start(xa_t[b4][:], xad[s]).then_inc(smp[s], 16)
                sync.dma_start(fbR[s % ND][:], fbd[s]).then_inc(fsem[s], 16)

            def emit_out(s):
                b = s % NB
                for jj in range(2):
                    sync.wait_ge(ssem, SSEM[(3, s, jj)])
                    sync.dma_start(
                        outr[s][:, 512 * jj : 512 * jj + 512],
                        ob[b][:, 512 * jj : 512 * jj + 512],
                    ).then_inc(osem[s], 16)

            # startup: xa0 first so the a-planes (and s1) start ASAP;
            # const tables go down the scalar engine's DMA queue in parallel
            sync.dma_start(xa_t[0][:], xad[0]).then_inc(smp[0], 16)
            sync.dma_start(fbR[0][:], fbd[0]).then_inc(fsem[0], 16)
            emit_in(1)
            for s_ in range(2, ND):
                emit_in(s_)
            for s in range(SPC):
                if s + ND < SPC:
                    emit_in(s + ND)
                emit_out(s)

        @block.gpsimd
        def _(gp):
            for kind, s in gp_order:
                b4 = s % ND
                gp.wait_ge(smp[s], 16)
                if s >= ND:
                    gp.wait_ge(psem, PSEM[(0, s - ND, 1)])  # A_t[b4] free
                xv = xa_t[b4][:, 0:1024].rearrange("p (c n) -> p c n", c=C)
                nc.gpsimd.tensor_tensor(
                    A_t[b4][:, 0:1024].rearrange("p (c n) -> p c n", c=C),
                    xv,
                    bcast8(xa_t[b4][:, 1024:1152]),
                    AluOp.mult,
                ).then_inc(gsem, 1)
                nc.gpsimd.tensor_tensor(
                    A_t[b4][:, 1024:2048].rearrange("p (c n) -> p c n", c=C),
                    xv,
                    bcast8(xa_t[b4][:, 1152:1280]),
                    AluOp.mult,
                ).then_inc(gsem, 1)

        @block.vector
        def _(vector):
            def chpages(ap):
                v_ = ap.rearrange("p (c u) -> p c u", c=C)
                return v_[:, :, 0:128], v_[:, :, 128:256]

            def prpages(ap):
                # s2 output layout: 4 q-blocks of (re 2ch*128 | im 2ch*128)
                v_ = ap.rearrange("p (q r u) -> p q r u", q=4, r=2)
                return v_[:, :, 0, :], v_[:, :, 1, :]  # [128, 4, 256] each

            def flat8(ap):
                return ap.rearrange("p (c u) -> p c u", c=C)

            def flat4(ap):
                return ap.rearrange("p (q u) -> p q u", q=4)

            def bcast4x2(tab):
                # [128,128] -> [128, 4, 2, 128] for the q-block × 2ch layout
                return tab.rearrange("p (o q u) -> p o q u", o=1, q=1).broadcast_to(
                    (128, 4, 2, 128)
                )

            def bch4(tab):
                return tab.rearrange("p (o u) -> p o u", o=1).broadcast_to(
                    (64, 4, 128)
                )

            for s0 in (0, 1):
                vector.wait_ge(smp[s0], 16)
                xv = xa_t[s0][:, 0:1024].rearrange("p (c n) -> p c n", c=C)
                for hh in range(2):
                    c4 = slice(4 * hh, 4 * hh + 4)
                    nc.vector.tensor_tensor(
                        A_t[s0][:, 0:1024].rearrange("p (c n) -> p c n", c=C)[
                            :, c4, :
                        ],
                        xv[:, c4, :],
                        bch4(xa_t[s0][:, 1024:1152]),
                        AluOp.mult,
                    ).then_inc(vsem, 1)
                    nc.vector.tensor_tensor(
                        A_t[s0][:, 1024:2048].rearrange("p (c n) -> p c n", c=C)[
                            :, c4, :
                        ],
                        xv[:, c4, :],
                        bch4(xa_t[s0][:, 1152:1280]),
                        AluOp.mult,
                    ).then_inc(vsem, 1)
            def bc_h(tab, n, w):
                # [128,w] table -> [128, n, w] zero-stride broadcast
                return tab.rearrange("p (o u) -> p o u", o=1).broadcast_to(
                    (128, n, w)
                )

            first_dve = [True]
            for grp, s in dve_order:
                if first_dve[0]:
                    vector.wait_ge(csem, 16)
                    first_dve[0] = False
                    first_l3 = [True]
                b = s % NB
                if grp == 0:
                    # L1 (fwd twiddle, Karatsuba planes) from Yf, by halves
                    if s >= NB:
                        vector.wait_ge(psem, PSEM[(1, s - NB, 1)])  # bufs free
                    yv = Yf[b][:].rearrange("p (c u) -> p c u", c=C)
                    pv = PQ1[b][:].rearrange("p (c u) -> p c u", c=C)
                    for h in range(2):
                        vector.wait_ge(ssem, SSEM[(0, s, h)])
                        c4 = slice(4 * h, 4 * h + 4)
                        nc.vector.tensor_tensor(
                            pv[:, c4, :], yv[:, c4, :],
                            bc_h(ca[:, 0:256], 4, 256), AluOp.mult,
                        ).then_inc(vsem, 1)  # [P|Qn] per channel
                        nc.vector.tensor_tensor(
                            flat8(S1[b][:])[:, c4, :],
                            yv[:, c4, 0:128], yv[:, c4, 128:256], AluOp.add,
                        ).then_inc(vsem, 1)
                        vector.wait_ge(vsem, VOP[(grp, s, h, 2)])  # S1h drained
                        nc.vector.tensor_tensor(
                            flat8(Rb[b][:])[:, c4, :],
                            flat8(S1[b][:])[:, c4, :],
                            bc_h(ca[:, _WS : _WS + 128], 4, 128), AluOp.mult,
                        ).then_inc(vsem, 1)
                elif grp == 1:
                    # C-layer (Fa o Fb, Karatsuba planes) from Ff, by halves
                    vector.wait_ge(fsem[s], 16)
                    if s >= NB:
                        vector.wait_ge(psem, PSEM[(2, s - NB, 1)])  # bufs free
                    fv = Ff[b][:].rearrange("p (q u) -> p q u", q=4)
                    cv = CRI[b][:].rearrange("p (q u) -> p q u", q=4)
                    for h in range(2):
                        vector.wait_ge(ssem, SSEM[(1, s, h)])
                        q2 = slice(2 * h, 2 * h + 2)
                        nc.vector.tensor_tensor(
                            cv[:, q2, :], fv[:, q2, :],
                            bc_h(fbR[s % ND][:, 0:512], 2, 512), AluOp.mult,
                        ).then_inc(vsem, 1)  # [CR 2ch | CI 2ch] per q
                        nc.vector.tensor_tensor(
                            flat4(M1[b][:])[:, q2, :],
                            fv[:, q2, 0:256], fv[:, q2, 256:512], AluOp.add,
                        ).then_inc(vsem, 1)
                        vector.wait_ge(vsem, VOP[(grp, s, h, 2)])  # M1h drained
                        nc.vector.tensor_tensor(
                            flat4(M2[b][:])[:, q2, :].rearrange(
                                "p q (c u) -> p q c u", c=2
                            ),
                            flat4(M1[b][:])[:, q2, :].rearrange(
                                "p q (c u) -> p q c u", c=2
                            ),
                            fbR[s % ND][:, 512:640].rearrange(
                                "p (o q u) -> p o q u", o=1, q=1
                            ).broadcast_to((128, 2, 2, 128)),
                            AluOp.mult,
                        ).then_inc(vsem, 1)
                else:
                    # L3 (inv twiddle, Karatsuba planes) from Sf, by halves
                    if first_l3[0]:
                        vector.wait_ge(c2sem, 16)
                        first_l3[0] = False
                    if s >= NB:
                        vector.wait_ge(psem, PSEM[(3, s - NB, 1)])  # bufs free
                    sv = Sf[b][:].rearrange("p (c u) -> p c u", c=C)
                    p3v = PQ3[b][:].rearrange("p (c u) -> p c u", c=C)
                    for h in range(2):
                        vector.wait_ge(ssem, SSEM[(2, s, h)])
                        c4 = slice(4 * h, 4 * h + 4)
                        nc.vector.tensor_tensor(
                            p3v[:, c4, :], sv[:, c4, :],
                            bc_h(ca[:, _W2R : _W2R + 256], 4, 256), AluOp.mult,
                        ).then_inc(vsem, 1)
                        nc.vector.tensor_tensor(
                            flat8(S3[b][:])[:, c4, :],
                            sv[:, c4, 0:128], sv[:, c4, 128:256], AluOp.add,
                        ).then_inc(vsem, 1)
                        vector.wait_ge(vsem, VOP[(grp, s, h, 2)])  # S3h drained
                        nc.vector.tensor_tensor(
                            flat8(R3b[b][:])[:, c4, :],
                            flat8(S3[b][:])[:, c4, :],
                            bc_h(ca[:, _W2S : _W2S + 128], 4, 128), AluOp.mult,
                        ).then_inc(vsem, 1)

        @block.tensor
        def _(tensor):
            mm = nc.tensor.matmul
            first_pe = [True]
            first_is1 = [True]

            def phase_s1(s, h):
                rg = psR[s % 2][:, 1024 * h : 1024 * h + 1024]
                if first_pe[0]:
                    tensor.wait_ge(cbsem, 16)  # cb loaded
                    first_pe[0] = False
                if s < 2:
                    tensor.wait_ge(vsem, 4 * s + 2 * (h + 1))  # startup planes
                elif h == 0:
                    tensor.wait_ge(gsem, GSEM[(0, s)])
                b4 = s % ND
                # is2(s-NB) left only cols [0:512] of this region occupied;
                # channels mapping to the free bank [512:1024] run first, and
                # the ob-evac wait guards only the conflicting bank.
                for c in (4 * h + 2, 4 * h + 3, 4 * h, 4 * h + 1):
                    if c == 4 * h and s >= NB:
                        tensor.wait_ge(ssem, SSEM[(3, s - NB, h)])
                    o = rg[:, 256 * (c - 4 * h) : 256 * (c - 4 * h) + 256]
                    mm(
                        o,
                        A_t[b4][:, 128 * c : 128 * c + 128],
                        cb[:, 0:256],
                        start=True,
                        stop=False,
                    )
                    i = mm(
                        o,
                        A_t[b4][:, 1024 + 128 * c : 1024 + 128 * c + 128],
                        cb[:, 256:512],
                        start=False,
                        stop=True,
                    )
                    if c % 4 == 1:
                        i.then_inc(psem, 1)

            def phase_s2(s, h):
                b = s % NB
                rg = psR[s % 2][:, 1024 * h : 1024 * h + 1024]
                if h == 0:
                    tensor.wait_ge(csem, 16)  # ca chunk1 loaded
                def pq1(q, r):
                    # plane r (0=P, 1=Qn) of channels 2q, 2q+1 from PQ1
                    return PQ1[b][:, 512 * q : 512 * q + 512].rearrange(
                        "p (c r u) -> p c r u", c=2, r=2
                    )[:, :, r, :]

                srcs = [
                    (0, _F[0], 0, True, False, 1),
                    (1, _F[0], 256, False, False, None),
                    (1, _F[1], 0, False, False, None),
                    (0, _F[2], 256, False, False, None),
                    (2, _F[3], 0, False, False, 3),
                    (2, _F[4], 256, False, True, None),
                ]
                for wi, (pr, fofs, oofs, st, sp, wk) in enumerate(srcs):
                    if wk is not None:
                        tensor.wait_ge(vsem, VOP[(0, s, h, wk)])
                    for ql in range(2):
                        q = 2 * h + ql
                        rhs = (
                            Rb[b][:, 256 * q : 256 * q + 256]
                            if pr == 2
                            else pq1(q, pr)
                        )
                        i = mm(
                            rg[:, 512 * ql + oofs : 512 * ql + oofs + 256],
                            ca[:, fofs : fofs + 128],
                            rhs,
                            start=st,
                            stop=sp,
                        )
                        if wi == 5 and ql == 1:
                            i.then_inc(psem, 1)

            def phase_is1(s, h):
                b = s % NB
                rg = psR[s % 2][:, 1024 * h : 1024 * h + 1024]
                if first_is1[0]:
                    tensor.wait_ge(c2sem, 16)  # H tables in the 2nd const DMA
                    first_is1[0] = False
                cs = range(4 * h, 4 * h + 4)
                tensor.wait_ge(vsem, VOP[(1, s, h, 1)])  # [CR|CI]h ready
                for c in cs:
                    # even channel opens its bank; odd writes the other half
                    mm(
                        rg[:, 256 * (c % 4) : 256 * (c % 4) + 256],
                        CRI[b][
                            :,
                            512 * (c // 2)
                            + 128 * (c % 2) : 512 * (c // 2)
                            + 128 * (c % 2)
                            + 128,
                        ],
                        ca[:, _HA : _HA + 256],
                        start=(c % 2 == 0),
                        stop=False,
                    )
                for c in cs:
                    mm(
                        rg[:, 256 * (c % 4) : 256 * (c % 4) + 256],
                        CRI[b][
                            :,
                            512 * (c // 2)
                            + 256
                            + 128 * (c % 2) : 512 * (c // 2)
                            + 256
                            + 128 * (c % 2)
                            + 128,
                        ],
                        ca[:, _HB : _HB + 256],
                        start=False,
                        stop=False,
                    )
                tensor.wait_ge(vsem, VOP[(1, s, h, 3)])  # M2h ready
                for c in cs:
                    i = mm(
                        rg[:, 256 * (c % 4) : 256 * (c % 4) + 256],
                        M2[b][:, 128 * c : 128 * c + 128],
                        ca[:, _HR : _HR + 256],
                        start=False,
                        stop=(c % 2 == 1),
                    )
                    if c % 4 == 3:
                        i.then_inc(psem, 1)

            def phase_is2(s, h):
                b = s % NB
                rg = psR[s % 2][:, 1024 * h : 1024 * h + 1024]
                cs = range(4 * h, 4 * h + 4)
                tensor.wait_ge(vsem, VOP[(2, s, h, 1)])  # [P3|Q3n]h ready
                for c in cs:
                    mm(
                        rg[:, 128 * (c % 4) : 128 * (c % 4) + 128],
                        PQ3[b][:, 256 * c : 256 * c + 128],
                        ca[:, _KP : _KP + 128],
                        start=(c % 4 == 0),
                        stop=False,
                    )
                for c in cs:
                    mm(
                        rg[:, 128 * (c % 4) : 128 * (c % 4) + 128],
                        PQ3[b][:, 256 * c + 128 : 256 * c + 256],
                        ca[:, _KQ : _KQ + 128],
                        start=False,
                        stop=False,
                    )
                tensor.wait_ge(vsem, VOP[(2, s, h, 3)])  # R3bh ready
                for c in cs:
                    i = mm(
                        rg[:, 128 * (c % 4) : 128 * (c % 4) + 128],
                        R3b[b][:, 128 * c : 128 * c + 128],
                        ca[:, _KR : _KR + 128],
                        start=False,
                        stop=(c % 4 == 3),
                    )
                    if c % 4 == 3:
                        i.then_inc(psem, 1)

            phase_fns = [phase_s1, phase_s2, phase_is1, phase_is2]
            for ph, s, h in pe_order:
                phase_fns[ph](s, h)

        @block.scalar
        def _(scalar):
            nc.scalar.dma_start(cb[:], cbd[:]).then_inc(cbsem, 16)
            nc.scalar.dma_start(ca[:, 0:CA1_COLS], cad[:, 0:CA1_COLS]).then_inc(
                csem, 16
            )
            nc.scalar.dma_start(ca[:, CA1_COLS:], cad[:, CA1_COLS:]).then_inc(
                c2sem, 16
            )
            for ph, s, h in act_order:
                b = s % NB
                ps = psR[s % 2]
                scalar.wait_ge(psem, PSEM[(ph, s, h)])
                if ph == 3:
                    if s >= NB and h == 0:
                        scalar.wait_ge(osem[s - NB], 32)
                    nc.scalar.copy(
                        ob[b][:, 512 * h : 512 * h + 512],
                        ps[:, 1024 * h : 1024 * h + 512],
                    ).then_inc(ssem, 1)
                else:
                    dst = [Yf, Ff, Sf][ph][b]
                    o = slice(1024 * h, 1024 * h + 1024)
                    nc.scalar.copy(dst[:, o], ps[:, o]).then_inc(ssem, 1)

    for t in reversed(ctx_list):
        t.__exit__(None, None, None)

    return nc


def _get_module():
    if "nc" not in _MODULE_CACHE:
        _MODULE_CACHE["nc"] = _build_module()
    return _MODULE_CACHE["nc"]


# ---------------------------------------------------------------------------
# host side
# ---------------------------------------------------------------------------


def _host_tables(rpm):
    """Per-sample chirp tables + Fb planes (un-replicated)."""
    pad = np.floor((RES * 60.0 / rpm.astype(np.float64) - TS) * SF).astype(np.int64)
    n_arr = L + pad
    t = np.arange(L, dtype=np.int64)
    m = np.arange(M, dtype=np.int64)
    mm = np.minimum(m, M - m)

    ach = np.empty((B, 256), np.float16)   # per n2-row: [cos 128 | -sin 128]
    fbp = np.empty((B, 128, 640), np.float16)
    for b in range(B):
        n = int(n_arr[b])
        two_n = 2 * n
        ph = np.pi * ((t * t) % two_n) / n
        cosv = np.cos(ph).astype(np.float16).reshape(64, 128)
        nsin = (-np.sin(ph)).astype(np.float16).reshape(64, 128)
        ach[b] = 0  # unused filler; real packing happens in kernel()
        _ACH_COS[b] = cosv
        _ACH_NSIN[b] = nsin
        phb = np.pi * ((mm * mm) % two_n) / n
        Fb = np.fft.fft(np.exp(1j * phb)).reshape(128, 128) * FBSCALE
        fr = Fb.real.astype(np.float16)
        fni = (-Fb.imag).astype(np.float16)
        fs = (Fb.real + Fb.imag).astype(np.float16)
        fbp[b] = np.concatenate([fr, fr, fni, fni, fs], axis=1)
    return fbp


_ACH_COS = np.empty((B, 64, 128), np.float16)
_ACH_NSIN = np.empty((B, 64, 128), np.float16)


LAST_EXEC_WALL_NS = [None]


def kernel(inputs, rpm):
    inputs = np.ascontiguousarray(inputs, dtype=np.float32)  # [B, L, C]
    rpm = np.ascontiguousarray(rpm, dtype=np.float32)

    ca, cb = _consts()
    fbp = _host_tables(rpm)
    # pack per-sample [64, 1280]: x as [n2, c, n1] cols 0:1024, chirp cols
    # 1024:1280 = [cos | -sin]
    xa = np.empty((B, 64, 1280), np.float16)
    xv = inputs.reshape(B, 64, 128, C).transpose(0, 1, 3, 2)  # [B, n2, c, n1]
    xa[:, :, 0:1024] = xv.reshape(B, 64, 1024).astype(np.float16)
    xa[:, :, 1024:1152] = _ACH_COS
    xa[:, :, 1152:1280] = _ACH_NSIN

    nc = _get_module()
    in_maps = []
    for g in range(NCORES):
        s0 = g * SPC
        in_maps.append(
            {
                "xad": xa[s0 : s0 + SPC],
                "fbd": fbp[s0 : s0 + SPC],
                "cad": ca,
                "cbd": cb,
            }
        )

    import time

    from concourse.bass_utils import run_bass_kernel_spmd

    t0 = time.perf_counter_ns()
    res = run_bass_kernel_spmd(nc, in_maps, list(range(NCORES)))
    LAST_EXEC_WALL_NS[0] = time.perf_counter_ns() - t0

    out = np.empty((B, L, C), np.float32)
    for g in range(NCORES):
        planes = np.asarray(res.results[g]["outr"], np.float32)  # [SPC, 128, 1024]
        arr = planes.reshape(SPC, 128, C, 2, 64)  # [s, m1, c, re|im, m2]
        mag = np.hypot(arr[:, :, :, 0, :], arr[:, :, :, 1, :])  # [s, m1, c, m2]
        # conv index k = m1 + 128*m2  ->  out[s, k, c]
        out[g * SPC : (g + 1) * SPC] = (
            mag.transpose(0, 3, 1, 2).reshape(SPC, L, C)
        )
    return out
